# revision 14
# baseline (speedup 1.0000x reference)
"""Trainium2 Bass kernel for a 2-layer GAT (B=16, N=1024, F=128, H=256).

Math (matching the reference):
  layer1: h1 = x @ W1; e = leakyrelu(src_i + dst_j) masked; attn = softmax(e)
          g = elu(attn @ h1); ln = LayerNorm(g) * gamma + beta
  layer2: out[:, :, -1] = elu(attn2 @ h2)[:, :, -1] + x[:, :, -1]
          where h2 = ln @ W2 -- only the last column of (attn2 @ h2) is
          needed, so layer 2 collapses to three projections of ln:
            src2 = ln @ (W2 a2[:F]), dst2 = ln @ (W2 a2[F:]),
            hcol = ln @ W2[:, -1]
          out_i = elu( sum_j P2_ij hcol_j / sum_j P2_ij ) + x_i,last

Key trick: exp(leakyrelu(s+d)) = max(exp(s)exp(d), exp(.2s)exp(.2d)) since
exp is monotonic -- so the NxN tiles need no transcendentals, just a
per-partition scale multiply and a max.  Softmax needs no max-subtraction
(scores are O(1)), and masked entries are exact zeros via the 0/1 mask.

Sharding: data-parallel over batch, 2 batches per core, 8 cores.
"""
import numpy as np
import ml_dtypes

import concourse.bass as bass
import concourse.bacc as bacc
import concourse.tile as tile
import concourse.mybir as mybir
from concourse import bass_utils
from concourse.masks import make_identity

B, N, F, H = 16, 1024, 128, 256
NCORES = 8
BPC = B // NCORES          # batches per core
P = 128                    # partitions
NCH = N // P               # 8 chunks of 128 nodes
ALPHA = 0.2
EPS = 1e-5

F32 = mybir.dt.float32
F32R = mybir.dt.float32r
BF16 = mybir.dt.bfloat16
AL = mybir.AluOpType
AF = mybir.ActivationFunctionType


def _dram_bcast(row_ap, parts=P):
    """Partition-broadcast read AP for a [1, ...] DRAM row."""
    return bass.AP(tensor=row_ap.tensor, offset=row_ap.offset,
                   ap=[[0, parts]] + [list(p) for p in row_ap.ap][1:])


def build_program(S, C, mm_dt=F32, stop_stage=99):
    """Build the per-core Bass program. S, C: per-projection scalars baked in."""
    S1, S2, S3 = [float(v) for v in S]
    c1, c2, c3 = [float(v) for v in C]

    nc = bacc.Bacc("TRN2", target_bir_lowering=False, debug=False)

    xT_d = nc.dram_tensor("xT", [BPC, P, N], mm_dt, kind="ExternalInput").ap()
    xl_d = nc.dram_tensor("xlast", [BPC, N], F32, kind="ExternalInput").ap()
    adjT_d = nc.dram_tensor("adjT", [N, N], BF16, kind="ExternalInput").ap()
    w1e_d = nc.dram_tensor("w1e", [P, H + 2], mm_dt, kind="ExternalInput").ap()
    w1srep_d = nc.dram_tensor("w1srep", [P, P], mm_dt, kind="ExternalInput").ap()
    u_d = nc.dram_tensor("u", [3, H], F32, kind="ExternalInput").ap()
    out_d = nc.dram_tensor("out", [BPC, N], F32, kind="ExternalOutput").ap()
    scr_d = nc.dram_tensor("scr", [BPC, 2, N], F32, kind="Internal").ap()

    def r(ap):
        return ap

    with tile.TileContext(nc) as tc:
        with (
            tc.tile_pool(name="const", bufs=1) as constp,
            tc.tile_pool(name="perb", bufs=2) as perb,
            tc.tile_pool(name="l2row", bufs=1) as l2row,
            tc.tile_pool(name="rows", bufs=1) as rows,
            tc.tile_pool(name="ptp", bufs=12) as ptp,
            tc.tile_pool(name="work", bufs=2) as work,
            tc.tile_pool(name="small", bufs=2) as small,
            tc.tile_pool(name="mmps", bufs=2, space="PSUM") as mmps,
            tc.tile_pool(name="sb1ps", bufs=1, space="PSUM") as sb1p,
            tc.tile_pool(name="gps", bufs=2, space="PSUM") as gpsum,
            tc.tile_pool(name="ndps", bufs=1, space="PSUM") as ndps,
            tc.tile_pool(name="tpps", bufs=1, space="PSUM") as tpps,
        ):
            ident = constp.tile([P, P], F32)
            make_identity(nc, ident)
            eps_c = constp.tile([P, 1], F32)
            nc.vector.memset(eps_c, EPS)
            w1e = constp.tile([P, H + 2], mm_dt)
            nc.sync.dma_start(out=w1e, in_=w1e_d)
            w1srep = constp.tile([P, P], mm_dt)
            nc.sync.dma_start(out=w1srep, in_=w1srep_d)
            U = []
            for k in range(3):
                ut = constp.tile([P, H], F32, tag=f"u{k}")
                nc.sync.dma_start(out=ut, in_=_dram_bcast(u_d[k:k + 1, :]))
                U.append(ut)
            adjT = constp.tile([P, NCH, N], BF16, tag="adjT")
            adjT_v = adjT_d.rearrange("(c p) i -> c p i", p=P)
            for c in range(NCH):
                nc.sync.dma_start(out=adjT[:, c, :], in_=adjT_v[c])

            for b in range(BPC):
                # ---- load x^T and x[:, -1]
                xt = perb.tile([P, N], mm_dt, tag="xt")
                nc.sync.dma_start(out=xt, in_=xT_d[b])
                xlr = rows.tile([1, N], F32, tag="xlr")
                nc.sync.dma_start(out=xlr, in_=xl_d[b:b + 1, :])

                # ---- h1e_j = x_chunk @ [W1 | w1s | w1d], plus ones column
                h1sb = []
                for j in range(NCH):
                    ps = mmps.tile([P, H + 2], F32, tag="h1ps")
                    nc.tensor.matmul(ps, r(xt[:, j * P:(j + 1) * P]), r(w1e),
                                     start=True, stop=True)
                    ht = perb.tile([P, H + 3], mm_dt, tag=f"h1sb{j}")
                    nc.vector.tensor_copy(ht[:, 0:H + 2], ps)
                    nc.vector.memset(ht[:, H + 2:H + 3], 1.0)
                    h1sb.append(ht)

                # ---- src1 broadcast rows via rank-1 matmul, then exp
                Bt = perb.tile([P, N], F32, tag="B")
                B2t = perb.tile([P, N], F32, tag="B2")
                for hh in range(2):
                    sl = slice(hh * 512, (hh + 1) * 512)
                    ps = sb1p.tile([P, 512], F32, tag="sb1ps")
                    nc.tensor.matmul(ps, r(w1srep), r(xt[:, sl]),
                                     start=True, stop=True)
                    nc.scalar.activation(Bt[:, sl], ps, AF.Exp)
                    nc.scalar.activation(B2t[:, sl], ps, AF.Exp, scale=ALPHA)

                if stop_stage <= 1:
                    nc.sync.dma_start(out=out_d[b:b + 1, :], in_=Bt[0:1, :])
                    continue

                # ---- exp(dst1) columns
                At = small.tile([P, NCH, 2], F32, tag="At")
                for j in range(NCH):
                    dcol = h1sb[j][:, H + 1:H + 2]
                    nc.scalar.activation(At[:, j, 0:1], dcol, AF.Exp)
                    nc.scalar.activation(At[:, j, 1:2], dcol, AF.Exp, scale=ALPHA)

                # ---- layer-1 score tiles P^T[j, i]
                pts = []
                for j in range(NCH):
                    m1 = work.tile([P, N], F32, tag="m1")
                    nc.scalar.activation(m1, Bt, AF.Copy, scale=At[:, j, 0:1])
                    m2 = work.tile([P, N], F32, tag="m2")
                    nc.vector.scalar_tensor_tensor(m2, B2t, At[:, j, 1:2], m1,
                                                   op0=AL.mult, op1=AL.max)
                    pt = ptp.tile([P, N], mm_dt, tag="pt")
                    nc.gpsimd.tensor_tensor(pt, m2, adjT[:, j, :], op=AL.mult)
                    pts.append(pt)

                if stop_stage <= 2:
                    nc.sync.dma_start(out=out_d[b:b + 1, :], in_=pts[0][0:1, :])
                    continue

                # ---- G_i = sum_j P^T[j,i].T @ [h1 | . | . | 1]  (col 258 = D)
                T1 = small.tile([P, NCH], F32, tag="T1")
                T2 = small.tile([P, NCH], F32, tag="T2")
                T3 = small.tile([P, NCH], F32, tag="T3")
                MV8 = small.tile([P, NCH, 2], F32, tag="MV8")
                for i in range(NCH):
                    g_ps = gpsum.tile([P, H + 3], F32, tag="G")
                    for j in range(NCH):
                        nc.tensor.matmul(g_ps,
                                         r(pts[j][:, i * P:(i + 1) * P]),
                                         r(h1sb[j]),
                                         start=(j == 0), stop=(j == NCH - 1))
                    if stop_stage <= 3.1:
                        g = work.tile([P, H], F32, tag="g")
                        nc.vector.tensor_copy(g, g_ps[:, 0:H])
                        continue
                    drec = small.tile([P, 1], F32, tag="drec")
                    nc.vector.reciprocal(drec, g_ps[:, H + 2:H + 3])
                    # g = elu(G/D) = min(exp(G/D) - 1, relu(G)/D)
                    ex = work.tile([P, H], F32, tag="ex")
                    nc.scalar.activation(ex, g_ps[:, 0:H], AF.Exp,
                                         scale=drec[:, 0:1])
                    rl = work.tile([P, H], F32, tag="rl")
                    nc.vector.tensor_scalar(rl, g_ps[:, 0:H], 0.0, drec[:, 0:1],
                                            op0=AL.max, op1=AL.mult)
                    g = work.tile([P, H], F32, tag="g")
                    nc.vector.scalar_tensor_tensor(g, ex, -1.0, rl,
                                                   op0=AL.add, op1=AL.min)
                    if stop_stage <= 3.2:
                        continue
                    st6 = small.tile([P, 6], F32, tag="st6")
                    nc.vector.bn_stats(out=st6, in_=g)
                    nc.vector.bn_aggr(out=MV8[:, i, :], in_=st6)
                    if stop_stage <= 3.3:
                        continue
                    junk = work.tile([P, H], F32, tag="junk")
                    for Tt, ut in ((T1, U[0]), (T2, U[1]), (T3, U[2])):
                        nc.vector.scalar_tensor_tensor(
                            junk, g, 1.0, ut, op0=AL.mult, op1=AL.mult,
                            accum_out=Tt[:, i:i + 1])

                if stop_stage <= 3.3:
                    nc.sync.dma_start(out=out_d[b:b + 1, 0:H], in_=g[0:1, :])
                    continue
                if stop_stage <= 3:
                    nc.sync.dma_start(out=out_d[b:b + 1, 0:NCH], in_=T1[0:1, :])
                    continue

                # ---- LayerNorm affine + projections (column form [128, 8])
                sd = small.tile([P, NCH], F32, tag="sd")
                nc.scalar.activation(sd, MV8[:, :, 1], AF.Sqrt,
                                     bias=eps_c[:, 0:1])
                rstd = small.tile([P, NCH], F32, tag="rstd")
                nc.vector.reciprocal(rstd, sd)
                mu = MV8[:, :, 0]

                dh = small.tile([P, NCH], F32, tag="dh")        # src2 cols
                s2c = small.tile([P, NCH], F32, tag="s2c")      # dst2 cols
                hcones = small.tile([P, NCH, 2], mm_dt, tag="hcones")
                nc.vector.memset(hcones[:, :, 1:2], 1.0)
                for Tt, SK, cK, dst in ((T1, S1, c1, dh),
                                        (T2, S2, c2, s2c),
                                        (T3, S3, c3, None)):
                    ak = work.tile([P, NCH], F32, tag="ak")
                    nc.vector.scalar_tensor_tensor(ak, mu, -SK, Tt,
                                                   op0=AL.mult, op1=AL.add)
                    bk = work.tile([P, NCH], F32, tag="bk")
                    nc.vector.scalar_tensor_tensor(bk, ak, 0.0, rstd,
                                                   op0=AL.add, op1=AL.mult)
                    if dst is None:
                        nc.vector.tensor_scalar_add(hcones[:, :, 0:1],
                                                    bk.unsqueeze(2), cK)
                    else:
                        nc.vector.tensor_scalar_add(dst, bk, cK)

                # exp(dst2) columns
                Ad = small.tile([P, NCH, 2], F32, tag="Ad")
                nc.scalar.activation(Ad[:, :, 0:1], s2c.unsqueeze(2), AF.Exp)
                nc.scalar.activation(Ad[:, :, 1:2], s2c.unsqueeze(2), AF.Exp,
                                     scale=ALPHA)

                # ---- src2: transpose cols -> row, exp, DRAM-bounce broadcast
                tp_ps = tpps.tile([NCH, P], F32, tag="tp")
                nc.tensor.transpose(tp_ps, dh, ident)
                tpe = small.tile([NCH, P, 2], F32, tag="tpe")
                nc.scalar.activation(tpe[:, :, 0:1], tp_ps.unsqueeze(2), AF.Exp)
                nc.scalar.activation(tpe[:, :, 1:2], tp_ps.unsqueeze(2), AF.Exp,
                                     scale=ALPHA)
                scr_v = scr_d.rearrange("b r (c p) -> b r c p", p=P)
                nc.sync.dma_start(out=scr_v[b, 0], in_=tpe[:, :, 0])
                nc.sync.dma_start(out=scr_v[b, 1], in_=tpe[:, :, 1])
                Et = l2row.tile([P, N], F32, tag="E")
                nc.sync.dma_start(out=Et, in_=_dram_bcast(scr_d[b, 0:1, :]))
                E2t = l2row.tile([P, N], F32, tag="E2")
                nc.sync.dma_start(out=E2t, in_=_dram_bcast(scr_d[b, 1:2, :]))

                if stop_stage <= 4:
                    nc.sync.dma_start(out=out_d[b:b + 1, :], in_=Et[0:1, :])
                    continue

                # ---- layer-2 score tiles + num/den matmul
                nd_ps = ndps.tile([2, N], F32, tag="nd")
                for j in range(NCH):
                    n1 = work.tile([P, N], F32, tag="m1")
                    nc.scalar.activation(n1, Et, AF.Copy, scale=Ad[:, j, 0:1])
                    n2 = work.tile([P, N], F32, tag="m2")
                    nc.vector.scalar_tensor_tensor(n2, E2t, Ad[:, j, 1:2], n1,
                                                   op0=AL.mult, op1=AL.max)
                    p2 = work.tile([P, N], mm_dt, tag="p2")
                    nc.vector.scalar_tensor_tensor(p2, n2, 1.0, adjT[:, j, :],
                                                   op0=AL.mult, op1=AL.mult)
                    for hh in range(2):
                        sl = slice(hh * 512, (hh + 1) * 512)
                        nc.tensor.matmul(nd_ps[:, sl], r(hcones[:, j, :]),
                                         r(p2[:, sl]),
                                         start=(j == 0), stop=(j == NCH - 1))

                if stop_stage <= 5:
                    dbg = rows.tile([2, N], F32, tag="ndsb")
                    nc.vector.tensor_copy(dbg, nd_ps)
                    nc.sync.dma_start(out=out_d[b:b + 1, :], in_=dbg[0:1, :])
                    continue

                # ---- final: out = elu(num/den) + x_last
                ndsb = rows.tile([2, N], F32, tag="ndsb")
                nc.vector.tensor_copy(ndsb, nd_ps)
                den0 = rows.tile([1, N], F32, tag="den0")
                nc.sync.dma_start(out=den0, in_=ndsb[1:2, :])
                rec = rows.tile([1, N], F32, tag="rec")
                nc.vector.reciprocal(rec, den0)
                q = rows.tile([1, N], F32, tag="q")
                nc.vector.scalar_tensor_tensor(q, ndsb[0:1, :], 1.0, rec,
                                               op0=AL.mult, op1=AL.mult)
                ex3 = rows.tile([1, N], F32, tag="ex3")
                nc.scalar.activation(ex3, q, AF.Exp)
                r3 = rows.tile([1, N], F32, tag="r3")
                nc.scalar.activation(r3, q, AF.Relu)
                o3 = rows.tile([1, N], F32, tag="o3")
                nc.vector.scalar_tensor_tensor(o3, ex3, -1.0, r3,
                                               op0=AL.add, op1=AL.min)
                orow = rows.tile([1, N], F32, tag="orow")
                nc.gpsimd.tensor_tensor(orow, o3, xlr, op=AL.add)
                nc.sync.dma_start(out=out_d[b:b + 1, :], in_=orow)

    nc.finalize()
    return nc


def prepare_inputs(x, adj, W1, a1, gamma, beta, W2, a2):
    """Host-side preprocessing; returns (S, C, per-core in_maps)."""
    x = np.asarray(x, np.float32)
    adj = np.asarray(adj)
    W1 = np.asarray(W1, np.float32)
    a1 = np.asarray(a1, np.float32)
    gamma = np.asarray(gamma, np.float32)
    beta = np.asarray(beta, np.float32)
    W2 = np.asarray(W2, np.float32)
    a2 = np.asarray(a2, np.float32)

    w1s = W1 @ a1[:H, 0]
    w1d = W1 @ a1[H:, 0]
    W1e = np.ascontiguousarray(
        np.concatenate([W1, w1s[:, None], w1d[:, None]], axis=1))
    w1srep = np.ascontiguousarray(np.repeat(w1s[:, None], P, axis=1))
    v1 = W2 @ a2[:F, 0]
    v2 = W2 @ a2[F:, 0]
    v3 = W2[:, F - 1]
    U = np.ascontiguousarray(np.stack([gamma * v1, gamma * v2, gamma * v3]))
    S = U.sum(axis=1)
    C = np.array([beta @ v1, beta @ v2, beta @ v3], np.float32)

    xT = np.ascontiguousarray(x.transpose(0, 2, 1))
    xlast = np.ascontiguousarray(x[:, :, F - 1])
    adjT = np.ascontiguousarray((adj.T > 0).astype(ml_dtypes.bfloat16))

    in_maps = []
    for c in range(NCORES):
        sl = slice(c * BPC, (c + 1) * BPC)
        in_maps.append({
            "xT": np.ascontiguousarray(xT[sl]),
            "xlast": np.ascontiguousarray(xlast[sl]),
            "adjT": adjT,
            "w1e": W1e,
            "w1srep": w1srep,
            "u": U,
        })
    return S, C, in_maps


def kernel(**inputs):
    S, C, in_maps = prepare_inputs(
        inputs["x"], inputs["adj"], inputs["W1"], inputs["a1"],
        inputs["gamma"], inputs["beta"], inputs["W2"], inputs["a2"])
    nc = build_program(S, C)
    res = bass_utils.run_bass_kernel_spmd(nc, in_maps,
                                          core_ids=list(range(NCORES)))
    out = np.concatenate([res.results[c]["out"] for c in range(NCORES)],
                         axis=0)
    return np.ascontiguousarray(out.astype(np.float32))


# revision 19
# speedup vs baseline: 1.1744x; 1.1744x over previous
"""Trainium2 Bass kernel for a 2-layer GAT (B=16, N=1024, F=128, H=256).

Math (matching the reference):
  layer1: h1 = x @ W1; e = leakyrelu(src_i + dst_j) masked; attn = softmax(e)
          g = elu(attn @ h1); ln = LayerNorm(g) * gamma + beta
  layer2: out[:, :, -1] = elu(attn2 @ h2)[:, :, -1] + x[:, :, -1]
          where h2 = ln @ W2 -- only the last column of (attn2 @ h2) is
          needed, so layer 2 collapses to three projections of ln:
            src2 = ln @ (W2 a2[:F]), dst2 = ln @ (W2 a2[F:]),
            hcol = ln @ W2[:, -1]
          out_i = elu( sum_j P2_ij hcol_j / sum_j P2_ij ) + x_i,last

Key trick: exp(leakyrelu(s+d)) = max(exp(s)exp(d), exp(.2s)exp(.2d)) since
exp is monotonic -- so the NxN tiles need no transcendentals, just a
per-partition scale multiply and a max.  Softmax needs no max-subtraction
(scores are O(1)), and masked entries are exact zeros via the 0/1 mask.

Sharding: data-parallel over batch, 2 batches per core, 8 cores.
"""
import numpy as np
import ml_dtypes

import concourse.bass as bass
import concourse.bacc as bacc
import concourse.tile as tile
import concourse.mybir as mybir
from concourse import bass_utils
from concourse.masks import make_identity

B, N, F, H = 16, 1024, 128, 256
NCORES = 8
BPC = B // NCORES          # batches per core
P = 128                    # partitions
NCH = N // P               # 8 chunks of 128 nodes
ALPHA = 0.2
EPS = 1e-5

F32 = mybir.dt.float32
F32R = mybir.dt.float32r
BF16 = mybir.dt.bfloat16
AL = mybir.AluOpType
AF = mybir.ActivationFunctionType


def _dram_bcast(row_ap, parts=P):
    """Partition-broadcast read AP for a [1, ...] DRAM row."""
    return bass.AP(tensor=row_ap.tensor, offset=row_ap.offset,
                   ap=[[0, parts]] + [list(p) for p in row_ap.ap][1:])


def build_program(S, C, mm_dt=F32R, stop_stage=99):
    """Build the per-core Bass program. S, C: per-projection scalars baked in."""
    S1, S2, S3 = [float(v) for v in S]
    c1, c2, c3 = [float(v) for v in C]

    nc = bacc.Bacc("TRN2", target_bir_lowering=False, debug=False)

    xT_d = nc.dram_tensor("xT", [BPC, P, N], mm_dt, kind="ExternalInput").ap()
    xl_d = nc.dram_tensor("xlast", [BPC, P, NCH], F32, kind="ExternalInput").ap()
    adjT_d = nc.dram_tensor("adjT", [N, N], BF16, kind="ExternalInput").ap()
    w1e_d = nc.dram_tensor("w1e", [P, H + 2], mm_dt, kind="ExternalInput").ap()
    w1srep_d = nc.dram_tensor("w1srep", [P, P], mm_dt, kind="ExternalInput").ap()
    u_d = nc.dram_tensor("u", [3, H], F32, kind="ExternalInput").ap()
    out_d = nc.dram_tensor("out", [BPC, N], F32, kind="ExternalOutput").ap()
    scr_d = nc.dram_tensor("scr", [BPC, 2, N], F32, kind="Internal").ap()

    def r(ap):
        return ap

    with tile.TileContext(nc) as tc:
        with (
            tc.tile_pool(name="const", bufs=1) as constp,
            tc.tile_pool(name="perb", bufs=2) as perb,
            tc.tile_pool(name="l2row", bufs=1) as l2row,
            tc.tile_pool(name="rows", bufs=1) as rows,
            tc.tile_pool(name="ptp", bufs=12) as ptp,
            tc.tile_pool(name="work", bufs=2) as work,
            tc.tile_pool(name="small", bufs=2) as small,
            tc.tile_pool(name="mmps", bufs=2, space="PSUM") as mmps,
            tc.tile_pool(name="sb1ps", bufs=1, space="PSUM") as sb1p,
            tc.tile_pool(name="gps", bufs=2, space="PSUM") as gpsum,
            tc.tile_pool(name="ndps", bufs=1, space="PSUM") as ndps,
            tc.tile_pool(name="tpps", bufs=1, space="PSUM") as tpps,
        ):
            ident = constp.tile([P, P], F32)
            make_identity(nc, ident)
            eps_c = constp.tile([P, 1], F32)
            nc.vector.memset(eps_c, EPS)
            w1e = constp.tile([P, H + 2], mm_dt)
            nc.sync.dma_start(out=w1e, in_=w1e_d)
            w1srep = constp.tile([P, P], mm_dt)
            nc.sync.dma_start(out=w1srep, in_=w1srep_d)
            U = []
            for k in range(3):
                ut = constp.tile([P, H], F32, tag=f"u{k}")
                nc.sync.dma_start(out=ut, in_=_dram_bcast(u_d[k:k + 1, :]))
                U.append(ut)
            adjT = constp.tile([P, NCH, N], BF16, tag="adjT")
            adjT_v = adjT_d.rearrange("(c p) i -> c p i", p=P)
            for c in range(NCH):
                nc.sync.dma_start(out=adjT[:, c, :], in_=adjT_v[c])

            for b in range(BPC):
                # ---- load x^T and x[:, -1]
                xt = perb.tile([P, N], mm_dt, tag="xt")
                nc.sync.dma_start(out=xt, in_=xT_d[b])
                xlc = small.tile([P, NCH], F32, tag="xlc")
                nc.sync.dma_start(out=xlc, in_=xl_d[b])

                # ---- h1e_j = x_chunk @ [W1 | w1s | w1d], plus ones column
                h1sb = []
                for j in range(NCH):
                    ps = mmps.tile([P, H + 2], F32, tag="h1ps")
                    nc.tensor.matmul(ps, r(xt[:, j * P:(j + 1) * P]), r(w1e),
                                     start=True, stop=True)
                    ht = perb.tile([P, H + 4], mm_dt, tag=f"h1sb{j}")
                    nc.vector.tensor_copy(ht[:, 0:H + 2], ps)
                    nc.vector.tensor_scalar(ht[:, H + 2:H + 4], ht[:, 0:2], 0.0,
                                            1.0, op0=AL.mult, op1=AL.add)
                    h1sb.append(ht)

                # ---- src1 broadcast rows via rank-1 matmul, then exp
                Bt = perb.tile([P, N], F32, tag="B")
                B2t = perb.tile([P, N], F32, tag="B2")
                for hh in range(2):
                    sl = slice(hh * 512, (hh + 1) * 512)
                    ps = sb1p.tile([P, 512], F32, tag="sb1ps")
                    nc.tensor.matmul(ps, r(w1srep), r(xt[:, sl]),
                                     start=True, stop=True)
                    nc.scalar.activation(Bt[:, sl], ps, AF.Exp)
                    nc.scalar.activation(B2t[:, sl], ps, AF.Exp, scale=ALPHA)

                if stop_stage <= 1:
                    nc.sync.dma_start(out=out_d[b:b + 1, :], in_=Bt[0:1, :])
                    continue

                # ---- exp(dst1) columns
                At = small.tile([P, NCH, 2], F32, tag="At")
                for j in range(NCH):
                    dcol = h1sb[j][:, H + 1:H + 2]
                    nc.scalar.activation(At[:, j, 0:1], dcol, AF.Exp)
                    nc.scalar.activation(At[:, j, 1:2], dcol, AF.Exp, scale=ALPHA)

                # ---- layer-1 score tiles P^T[j, i]
                pts = []
                for j in range(NCH):
                    m1 = work.tile([P, N], F32, tag="m1")
                    nc.scalar.activation(m1, Bt, AF.Copy, scale=At[:, j, 0:1])
                    m2 = work.tile([P, N], F32, tag="m2")
                    nc.vector.scalar_tensor_tensor(m2, B2t, At[:, j, 1:2], m1,
                                                   op0=AL.mult, op1=AL.max)
                    pt = ptp.tile([P, N], mm_dt, tag="pt")
                    nc.gpsimd.tensor_tensor(pt, m2, adjT[:, j, :], op=AL.mult)
                    pts.append(pt)

                if stop_stage <= 2:
                    nc.sync.dma_start(out=out_d[b:b + 1, :], in_=pts[0][0:1, :])
                    continue

                # ---- G_i = sum_j P^T[j,i].T @ [h1 | . | . | 1]  (col 258 = D)
                T1 = small.tile([P, NCH], F32, tag="T1")
                T2 = small.tile([P, NCH], F32, tag="T2")
                T3 = small.tile([P, NCH], F32, tag="T3")
                MV8 = small.tile([P, NCH, 2], F32, tag="MV8")
                for i in range(NCH):
                    g_ps = gpsum.tile([P, H + 4], F32, tag="G")
                    for j in range(NCH):
                        nc.tensor.matmul(g_ps,
                                         r(pts[j][:, i * P:(i + 1) * P]),
                                         r(h1sb[j]),
                                         start=(j == 0), stop=(j == NCH - 1))
                    if stop_stage <= 3.1:
                        g = work.tile([P, H], F32, tag="g")
                        nc.vector.tensor_copy(g, g_ps[:, 0:H])
                        continue
                    drec = small.tile([P, 1], F32, tag="drec")
                    nc.vector.reciprocal(drec, g_ps[:, H + 2:H + 3])
                    # g = elu(G/D) = min(exp(G/D) - 1, relu(G)/D)
                    ex = work.tile([P, H], F32, tag="ex")
                    nc.scalar.activation(ex, g_ps[:, 0:H], AF.Exp,
                                         scale=drec[:, 0:1])
                    rl = work.tile([P, H], F32, tag="rl")
                    nc.vector.tensor_scalar(rl, g_ps[:, 0:H], 0.0, drec[:, 0:1],
                                            op0=AL.max, op1=AL.mult)
                    g = work.tile([P, H], F32, tag="g")
                    nc.vector.scalar_tensor_tensor(g, ex, -1.0, rl,
                                                   op0=AL.add, op1=AL.min)
                    if stop_stage <= 3.2:
                        continue
                    st6 = small.tile([P, 6], F32, tag="st6")
                    nc.vector.bn_stats(out=st6, in_=g)
                    nc.vector.bn_aggr(out=MV8[:, i, :], in_=st6)
                    if stop_stage <= 3.3:
                        continue
                    junk = work.tile([P, H], F32, tag="junk")
                    for Tt, ut in ((T1, U[0]), (T2, U[1]), (T3, U[2])):
                        nc.vector.scalar_tensor_tensor(
                            junk, g, 1.0, ut, op0=AL.mult, op1=AL.mult,
                            accum_out=Tt[:, i:i + 1])

                if stop_stage <= 3.3:
                    nc.sync.dma_start(out=out_d[b:b + 1, 0:H], in_=g[0:1, :])
                    continue
                if stop_stage <= 3:
                    nc.sync.dma_start(out=out_d[b:b + 1, 0:NCH], in_=T1[0:1, :])
                    continue

                # ---- LayerNorm affine + projections (column form [128, 8])
                sd = small.tile([P, NCH], F32, tag="sd")
                nc.scalar.activation(sd, MV8[:, :, 1], AF.Sqrt,
                                     bias=eps_c[:, 0:1])
                rstd = small.tile([P, NCH], F32, tag="rstd")
                nc.vector.reciprocal(rstd, sd)
                mu = MV8[:, :, 0]

                dh = small.tile([P, NCH], F32, tag="dh")        # src2 cols
                s2c = small.tile([P, NCH], F32, tag="s2c")      # dst2 cols
                hcones = small.tile([P, NCH, 2], mm_dt, tag="hcones")
                nc.vector.tensor_scalar(hcones[:, :, 1:2], MV8[:, :, 0:1], 0.0,
                                        1.0, op0=AL.mult, op1=AL.add)
                for Tt, SK, cK, dst in ((T1, S1, c1, dh),
                                        (T2, S2, c2, s2c),
                                        (T3, S3, c3, None)):
                    ak = work.tile([P, NCH], F32, tag="ak")
                    nc.vector.scalar_tensor_tensor(ak, mu, -SK, Tt,
                                                   op0=AL.mult, op1=AL.add)
                    bk = work.tile([P, NCH], F32, tag="bk")
                    nc.vector.scalar_tensor_tensor(bk, ak, 0.0, rstd,
                                                   op0=AL.add, op1=AL.mult)
                    if dst is None:
                        nc.vector.tensor_scalar_add(hcones[:, :, 0:1],
                                                    bk.unsqueeze(2), cK)
                    else:
                        nc.vector.tensor_scalar_add(dst, bk, cK)

                # exp(dst2) columns
                Ad = small.tile([P, NCH, 2], F32, tag="Ad")
                nc.scalar.activation(Ad[:, :, 0:1], s2c.unsqueeze(2), AF.Exp)
                nc.scalar.activation(Ad[:, :, 1:2], s2c.unsqueeze(2), AF.Exp,
                                     scale=ALPHA)

                # ---- src2: transpose cols -> row, exp, DRAM-bounce broadcast
                tp_ps = tpps.tile([NCH, P], F32, tag="tp")
                nc.tensor.transpose(tp_ps, dh, ident)
                tpe = small.tile([NCH, P, 2], F32, tag="tpe")
                nc.scalar.activation(tpe[:, :, 0:1], tp_ps.unsqueeze(2), AF.Exp)
                nc.scalar.activation(tpe[:, :, 1:2], tp_ps.unsqueeze(2), AF.Exp,
                                     scale=ALPHA)
                scr_v = scr_d.rearrange("b r (c p) -> b r c p", p=P)
                nc.sync.dma_start(out=scr_v[b, 0], in_=tpe[:, :, 0])
                nc.sync.dma_start(out=scr_v[b, 1], in_=tpe[:, :, 1])
                Et = l2row.tile([P, N], F32, tag="E")
                nc.sync.dma_start(out=Et, in_=_dram_bcast(scr_d[b, 0:1, :]))
                E2t = l2row.tile([P, N], F32, tag="E2")
                nc.sync.dma_start(out=E2t, in_=_dram_bcast(scr_d[b, 1:2, :]))

                if stop_stage <= 4:
                    nc.sync.dma_start(out=out_d[b:b + 1, :], in_=Et[0:1, :])
                    continue

                # ---- layer-2 score tiles + num/den matmul
                nd_ps = ndps.tile([2, N], F32, tag="nd")
                for j in range(NCH):
                    n1 = work.tile([P, N], F32, tag="m1")
                    nc.scalar.activation(n1, Et, AF.Copy, scale=Ad[:, j, 0:1])
                    n2 = work.tile([P, N], F32, tag="m2")
                    nc.vector.scalar_tensor_tensor(n2, E2t, Ad[:, j, 1:2], n1,
                                                   op0=AL.mult, op1=AL.max)
                    p2 = work.tile([P, N], mm_dt, tag="p2")
                    nc.vector.scalar_tensor_tensor(p2, n2, 1.0, adjT[:, j, :],
                                                   op0=AL.mult, op1=AL.mult)
                    for hh in range(2):
                        sl = slice(hh * 512, (hh + 1) * 512)
                        nc.tensor.matmul(nd_ps[:, sl], r(hcones[:, j, :]),
                                         r(p2[:, sl]),
                                         start=(j == 0), stop=(j == NCH - 1))

                if stop_stage <= 5:
                    dbg = rows.tile([2, N], F32, tag="ndsb")
                    nc.vector.tensor_copy(dbg, nd_ps)
                    nc.sync.dma_start(out=out_d[b:b + 1, :], in_=dbg[0:1, :])
                    continue

                # ---- final: out = elu(num/den) + x_last  (column form)
                ndsb = rows.tile([2, N], F32, tag="ndsb")
                nc.vector.tensor_copy(ndsb, nd_ps)
                ndc = small.tile([P, NCH, 2], F32, tag="ndc")
                for c in range(NCH):
                    tp2 = tpps.tile([P, 2], F32, tag="tp")
                    nc.tensor.transpose(tp2, ndsb[:, c * P:(c + 1) * P], ident[0:2, 0:2])
                    nc.vector.tensor_copy(ndc[:, c, :], tp2)
                recc = small.tile([P, NCH], F32, tag="recc")
                nc.vector.reciprocal(recc, ndc[:, :, 1])
                qc = small.tile([P, NCH], F32, tag="qc")
                nc.vector.scalar_tensor_tensor(qc, ndc[:, :, 0], 1.0, recc,
                                               op0=AL.mult, op1=AL.mult)
                exq = small.tile([P, NCH], F32, tag="exq")
                nc.scalar.activation(exq, qc, AF.Exp)
                rq = small.tile([P, NCH], F32, tag="rq")
                nc.scalar.activation(rq, qc, AF.Relu)
                oq = small.tile([P, NCH], F32, tag="oq")
                nc.vector.scalar_tensor_tensor(oq, exq, -1.0, rq,
                                               op0=AL.add, op1=AL.min)
                outc = small.tile([P, NCH], F32, tag="outc")
                nc.vector.scalar_tensor_tensor(outc, oq, 0.0, xlc,
                                               op0=AL.add, op1=AL.add)
                tpo = tpps.tile([NCH, P], F32, tag="tp")
                nc.tensor.transpose(tpo, outc, ident)
                osb = small.tile([NCH, P], F32, tag="osb")
                nc.vector.tensor_copy(osb, tpo)
                nc.sync.dma_start(
                    out=out_d.rearrange("b (c p) -> b c p", p=P)[b], in_=osb)

    nc.finalize()
    return nc


def prepare_inputs(x, adj, W1, a1, gamma, beta, W2, a2):
    """Host-side preprocessing; returns (S, C, per-core in_maps)."""
    x = np.asarray(x, np.float32)
    adj = np.asarray(adj)
    W1 = np.asarray(W1, np.float32)
    a1 = np.asarray(a1, np.float32)
    gamma = np.asarray(gamma, np.float32)
    beta = np.asarray(beta, np.float32)
    W2 = np.asarray(W2, np.float32)
    a2 = np.asarray(a2, np.float32)

    w1s = W1 @ a1[:H, 0]
    w1d = W1 @ a1[H:, 0]
    W1e = np.ascontiguousarray(
        np.concatenate([W1, w1s[:, None], w1d[:, None]], axis=1))
    w1srep = np.ascontiguousarray(np.repeat(w1s[:, None], P, axis=1))
    v1 = W2 @ a2[:F, 0]
    v2 = W2 @ a2[F:, 0]
    v3 = W2[:, F - 1]
    U = np.ascontiguousarray(np.stack([gamma * v1, gamma * v2, gamma * v3]))
    S = U.sum(axis=1)
    C = np.array([beta @ v1, beta @ v2, beta @ v3], np.float32)

    xT = np.ascontiguousarray(x.transpose(0, 2, 1))
    xlast = np.ascontiguousarray(
        x[:, :, F - 1].reshape(B, N // P, P).transpose(0, 2, 1))
    adjT = np.ascontiguousarray((adj.T > 0).astype(ml_dtypes.bfloat16))

    in_maps = []
    for c in range(NCORES):
        sl = slice(c * BPC, (c + 1) * BPC)
        in_maps.append({
            "xT": np.ascontiguousarray(xT[sl]),
            "xlast": np.ascontiguousarray(xlast[sl]),
            "adjT": adjT,
            "w1e": W1e,
            "w1srep": w1srep,
            "u": U,
        })
    return S, C, in_maps


def kernel(**inputs):
    S, C, in_maps = prepare_inputs(
        inputs["x"], inputs["adj"], inputs["W1"], inputs["a1"],
        inputs["gamma"], inputs["beta"], inputs["W2"], inputs["a2"])
    nc = build_program(S, C)
    res = bass_utils.run_bass_kernel_spmd(nc, in_maps,
                                          core_ids=list(range(NCORES)))
    out = np.concatenate([res.results[c]["out"] for c in range(NCORES)],
                         axis=0)
    return np.ascontiguousarray(out.astype(np.float32))
